# revision 11
# baseline (speedup 1.0000x reference)
"""Trainium2 Bass kernel for a pre-LN multi-head attention block.

Model (per batch b): LayerNorm(x) -> QKV -> 16-head attention (dh=64) ->
output projection + bias.

Sharding over 8 NeuronCores: core c handles batch b = c//2 and head group
g = c%2 (8 heads each).  LN is computed per-core on its batch (duplicated
across the pair).  The output projection produces partial sums over the
core's 512 inner dims; an on-device AllReduce over each pair {2b, 2b+1}
completes the projection.  Host folds ln_gamma/ln_beta into the QKV weights
and adds b_out at the end.

Device-side layout notes:
 - Activations are kept "transposed" (feature dim on partitions) so every
   matmul contracts over the partition axis.
 - Scores are computed directly as S^T [nk, nq]; softmax needs no max
   subtraction here (scores ~ N(0,1)), so exp is a single ScalarE pass and
   the denominator comes from a ones-column appended to V in the PV matmul.
"""

import numpy as np
from ml_dtypes import bfloat16

B, N, D = 4, 2048, 1024
HEADS, DH = 16, 64
SCALE = DH ** -0.5
NCORES = 8
HPC = 8                     # heads per core
GI = HPC * DH               # 512: inner dims per core
QR = 3 * GI                 # 1536: qkv rows per core
EPS = 1e-5
NT = N // 128               # 16 sequence tiles
KD = D // 128               # 8 feature tiles
NO = QR // 128              # 12 qkv output tiles
NKT = N // 128              # 16 key tiles

_cache = {}


def _build(use_cc=True):
    import concourse.bass as bass
    import concourse.mybir as mybir
    import concourse.bacc as bacc
    import concourse.tile as tile
    from concourse.masks import make_identity

    f32 = mybir.dt.float32
    bf16 = mybir.dt.bfloat16
    AX = mybir.AxisListType
    ALU = mybir.AluOpType
    ACTF = mybir.ActivationFunctionType

    nc = bacc.Bacc(
        "TRN2",
        target_bir_lowering=False,
        debug=False,
        enable_asserts=True,
        num_devices=NCORES,
    )

    x_d = nc.dram_tensor("x", [N, D], f32, kind="ExternalInput").ap()
    wq_d = nc.dram_tensor("wqkvT", [D, QR], bf16, kind="ExternalInput").ap()
    bias_d = nc.dram_tensor("qkv_bias", [128, NO], f32, kind="ExternalInput").ap()
    wo_d = nc.dram_tensor("woutT", [GI, D], bf16, kind="ExternalInput").ap()
    out_d = nc.dram_tensor("out", [N, D], f32, kind="ExternalOutput").ap()
    bounce = nc.dram_tensor("cc_in", [N, D], f32, kind="Internal")
    cc_out = nc.dram_tensor("cc_out", [N, D], f32, kind="Internal")

    with tile.TileContext(nc) as tc:
        with (
            tc.tile_pool(name="persist", bufs=1) as P,
            tc.tile_pool(name="ppool", bufs=1, space="PSUM") as PS,
            tc.tile_pool(name="trans", bufs=1) as T,
        ):
            ident = P.tile([128, 128], bf16, name="ident", tag="ident")
            make_identity(nc, ident)

            bias_sb = P.tile([128, NO], f32, name="bias_sb", tag="bias_sb")
            nc.sync.dma_start(bias_sb, bias_d)

            wq_sb = []
            for k in range(KD):
                t = P.tile([128, QR], bf16, name=f"wq{k}", tag=f"wq{k}")
                nc.sync.dma_start(t, wq_d[k * 128:(k + 1) * 128, :])
                wq_sb.append(t)
            wo_sb = []
            for k in range(4):
                t = P.tile([128, D], bf16, name=f"wo{k}", tag=f"wo{k}")
                nc.sync.dma_start(t, wo_d[k * 128:(k + 1) * 128, :])
                wo_sb.append(t)

            # xnT: transposed normalized activations, [d, n] flattened as
            # [128, KD*N]; block kd lives at cols [kd*N, (kd+1)*N).
            xnT = P.tile([128, KD * N], bf16, name="xnT", tag="xnT")
            xnT3 = xnT.rearrange("p (k n) -> p k n", k=KD)
            # qkvT: [1536, 2048] as 12 partition tiles.
            qkvT = []
            for o in range(NO):
                qkvT.append(P.tile([128, N], bf16, name=f"qkvT{o}", tag=f"qkvT{o}"))
            # normalized attention outputs, transposed: [512, 2048] as 4 tiles
            onormT = []
            for k in range(4):
                onormT.append(P.tile([128, N], bf16, name=f"onormT{k}", tag=f"onormT{k}"))

            sq_scr = T.tile([128, D], f32, name="sq_scr", tag="sq", bufs=1)
            eps_t = P.tile([128, 1], f32, name="eps_t", tag="eps_t")
            nc.vector.memset(eps_t, EPS)

            # ---- Phase A: LayerNorm + transpose, pipelined over seq tiles
            for nt in range(NT):
                x_t = T.tile([128, D], f32, name=f"x{nt}", tag="x", bufs=2)
                nc.sync.dma_start(x_t, x_d[nt * 128:(nt + 1) * 128, :])
                ssum = T.tile([128, 1], f32, name=f"ss{nt}", tag="ss", bufs=2)
                nc.vector.tensor_reduce(out=ssum, in_=x_t, axis=AX.X, op=ALU.add)
                mean = T.tile([128, 1], f32, name=f"mn{nt}", tag="mn", bufs=2)
                nc.scalar.mul(mean, ssum, 1.0 / D)
                xc = T.tile([128, D], f32, name=f"xc{nt}", tag="xc", bufs=2)
                nc.vector.tensor_scalar_sub(xc, x_t, mean)
                var = T.tile([128, 1], f32, name=f"vr{nt}", tag="vr", bufs=2)
                nc.scalar.activation(sq_scr, xc, ACTF.Square, accum_out=var)
                std = T.tile([128, 1], f32, name=f"st{nt}", tag="st", bufs=2)
                nc.scalar.activation(std, var, ACTF.Sqrt, bias=eps_t, scale=1.0 / D)
                rstd = T.tile([128, 1], f32, name=f"rs{nt}", tag="rs", bufs=2)
                nc.vector.reciprocal(rstd, std)
                xhat = T.tile([128, D], bf16, name=f"xh{nt}", tag="xh", bufs=2)
                nc.vector.tensor_scalar_mul(xhat, xc, rstd)
                for g2 in range(2):
                    tp = PS.tile([128, 512], bf16, name=f"tp{nt}_{g2}", tag="work", bufs=2)
                    for j in range(4):
                        kd = g2 * 4 + j
                        nc.tensor.transpose(
                            tp[:, j * 128:(j + 1) * 128],
                            xhat[:, kd * 128:(kd + 1) * 128],
                            ident,
                        )
                    dest = xnT3[:, g2 * 4:(g2 + 1) * 4, nt * 128:(nt + 1) * 128]
                    src = tp.rearrange("p (k n) -> p k n", k=4)
                    if (nt + g2) % 2 == 0:
                        nc.vector.tensor_copy(dest, src)
                    else:
                        nc.scalar.copy(dest, src)

            # ---- Phase B: QKV projection (heads 0,1 first so attention can start)
            o_order = [0, 4, 8, 1, 5, 9, 2, 6, 10, 3, 7, 11]
            for oi, o in enumerate(o_order):
                for c in range(4):
                    qp = PS.tile([128, 512], f32, name=f"qp{o}_{c}", tag="work", bufs=2)
                    for k in range(KD):
                        nc.tensor.matmul(
                            qp,
                            lhsT=wq_sb[k][:, o * 128:(o + 1) * 128],
                            rhs=xnT3[:, k, c * 512:(c + 1) * 512],
                            start=(k == 0),
                            stop=(k == KD - 1),
                        )
                    dst = qkvT[o][:, c * 512:(c + 1) * 512]
                    if (oi + c) % 2 == 0:
                        nc.scalar.activation(
                            dst, qp, ACTF.Identity, bias=bias_sb[:, o:o + 1]
                        )
                    else:
                        nc.vector.tensor_scalar_add(dst, qp, bias_sb[:, o:o + 1])

            # ---- Phase B': V transposed to natural layout + ones column
            vext = []
            for h in range(HPC):
                p0 = (h % 2) * 64
                vt = qkvT[8 + h // 2][p0:p0 + 64, :]
                id64 = ident[p0:p0 + 64, p0:p0 + 64]
                ve = P.tile([128, NKT * 65], bf16, name=f"vext{h}", tag=f"vext{h}")
                nc.vector.memset(ve, 1.0)
                ve3 = ve.rearrange("p (k e) -> p k e", e=65)
                for g2 in range(2):
                    tp = PS.tile([128, 512], bf16, name=f"vt{h}_{g2}", tag="work", bufs=2)
                    for j in range(8):
                        kt = g2 * 8 + j
                        nc.tensor.transpose(
                            tp[:, j * 64:(j + 1) * 64],
                            vt[:, kt * 128:(kt + 1) * 128],
                            id64,
                        )
                    dest = ve3[:, g2 * 8:(g2 + 1) * 8, 0:64]
                    src = tp.rearrange("p (k e) -> p k e", e=64)
                    if (h + g2) % 2 == 0:
                        nc.vector.tensor_copy(dest, src)
                    else:
                        nc.scalar.copy(dest, src)
                vext.append(ve3)

            # ---- Phase C: attention, S^T orientation, per (half, head)
            for half in range(2):
                for h in range(HPC):
                    q_t = qkvT[h // 2][(h % 2) * 64:(h % 2) * 64 + 64, :]
                    k_t = qkvT[4 + h // 2][(h % 2) * 64:(h % 2) * 64 + 64, :]
                    ops = PS.tile(
                        [65, 1024], f32, name=f"ops{half}_{h}", tag="acc", bufs=2
                    )
                    for kt in range(NKT):
                        sps = PS.tile(
                            [128, 1024], f32, name=f"s{half}_{h}_{kt}",
                            tag="work", bufs=2,
                        )
                        for c2 in range(2):
                            nc.tensor.matmul(
                                sps[:, c2 * 512:(c2 + 1) * 512],
                                lhsT=k_t[:, kt * 128:(kt + 1) * 128],
                                rhs=q_t[:, half * 1024 + c2 * 512:
                                        half * 1024 + (c2 + 1) * 512],
                                start=True,
                                stop=True,
                            )
                        pt = T.tile(
                            [128, 1024], bf16, name=f"pt{half}_{h}_{kt}",
                            tag="pt", bufs=2,
                        )
                        nc.scalar.activation(pt, sps, ACTF.Exp, scale=SCALE)
                        for c2 in range(2):
                            nc.tensor.matmul(
                                ops[:, c2 * 512:(c2 + 1) * 512],
                                lhsT=vext[h][:, kt, :],
                                rhs=pt[:, c2 * 512:(c2 + 1) * 512],
                                start=(kt == 0),
                                stop=(kt == NKT - 1),
                            )
                    # normalize by the softmax denominator (row 64 of ops)
                    rl = T.tile([1, 1024], f32, name=f"rl{half}_{h}", tag="rl", bufs=2)
                    nc.vector.reciprocal(rl, ops[64:65, :])
                    rlb = T.tile(
                        [64, 1024], f32, name=f"rlb{half}_{h}", tag="rlb", bufs=2
                    )
                    nc.gpsimd.partition_broadcast(rlb, rl, channels=64)
                    nc.vector.tensor_mul(
                        onormT[h // 2][(h % 2) * 64:(h % 2) * 64 + 64,
                                       half * 1024:(half + 1) * 1024],
                        ops[0:64, :],
                        rlb,
                    )

                # ---- Phase D: output projection for this half
                for ntl in range(NT // 2):
                    nt = half * (NT // 2) + ntl
                    po = T.tile([128, D], f32, name=f"po{nt}", tag="po", bufs=2)
                    for c in range(2):
                        pp = PS.tile(
                            [128, 512], f32, name=f"pp{nt}_{c}", tag="work", bufs=2
                        )
                        for kq in range(4):
                            nc.tensor.matmul(
                                pp,
                                lhsT=onormT[kq][:, nt * 128:(nt + 1) * 128],
                                rhs=wo_sb[kq][:, c * 512:(c + 1) * 512],
                                start=(kq == 0),
                                stop=(kq == 3),
                            )
                        if c == 0:
                            nc.scalar.copy(po[:, c * 512:(c + 1) * 512], pp)
                        else:
                            nc.vector.tensor_copy(po[:, c * 512:(c + 1) * 512], pp)
                    nc.sync.dma_start(bounce.ap()[nt * 128:(nt + 1) * 128, :], po)

            # ---- Phase E: pair AllReduce of projection partials
            if use_cc:
                nc.gpsimd.collective_compute(
                    "AllReduce",
                    ALU.add,
                    replica_groups=[[0, 1], [2, 3], [4, 5], [6, 7]],
                    ins=[bounce.ap()],
                    outs=[cc_out.ap()],
                )
                nc.sync.dma_start(out_d, cc_out.ap())
            else:
                nc.sync.dma_start(out_d, bounce.ap())

    nc.compile()
    return nc


def _shard_inputs(x, ln_gamma, ln_beta, w_qkv, w_out):
    w_eff = (w_qkv * ln_gamma[None, :]).astype(np.float32)
    in_maps = []
    for c in range(NCORES):
        b, g = c // 2, c % 2
        rows = np.r_[g * GI:(g + 1) * GI,
                     1024 + g * GI:1024 + (g + 1) * GI,
                     2048 + g * GI:2048 + (g + 1) * GI]
        w_c = w_eff[rows]                                    # [1536, 1024]
        wqkvT_c = np.ascontiguousarray(w_c.T).astype(bfloat16)
        bias_c = (w_qkv[rows].astype(np.float64) @ ln_beta.astype(np.float64))
        bias_2d = np.ascontiguousarray(
            bias_c.reshape(NO, 128).T
        ).astype(np.float32)                                 # [128, 12]
        woutT_c = np.ascontiguousarray(
            w_out[:, g * GI:(g + 1) * GI].T
        ).astype(bfloat16)                                   # [512, 1024]
        in_maps.append({
            "x": np.ascontiguousarray(x[b]).astype(np.float32),
            "wqkvT": wqkvT_c,
            "qkv_bias": bias_2d,
            "woutT": woutT_c,
        })
    return in_maps


def kernel(x, ln_gamma, ln_beta, w_qkv, w_out, b_out, _trace=False, _use_cc=True):
    from concourse import bass_utils

    x = np.asarray(x, dtype=np.float32)
    ln_gamma = np.asarray(ln_gamma, dtype=np.float32)
    ln_beta = np.asarray(ln_beta, dtype=np.float32)
    w_qkv = np.asarray(w_qkv, dtype=np.float32)
    w_out = np.asarray(w_out, dtype=np.float32)
    b_out = np.asarray(b_out, dtype=np.float32)

    key = ("nc", _use_cc)
    if key not in _cache:
        _cache[key] = _build(use_cc=_use_cc)
    nc = _cache[key]

    in_maps = _shard_inputs(x, ln_gamma, ln_beta, w_qkv, w_out)
    res = bass_utils.run_bass_kernel_spmd(
        nc, in_maps, core_ids=list(range(NCORES)), trace=_trace
    )
    out = np.empty((B, N, D), dtype=np.float32)
    for b in range(B):
        if _use_cc:
            out[b] = np.asarray(res.results[2 * b]["out"])
        else:
            out[b] = (np.asarray(res.results[2 * b]["out"])
                      + np.asarray(res.results[2 * b + 1]["out"]))
    out += b_out[None, None, :]
    _cache["last_result"] = res
    return out


# revision 12
# speedup vs baseline: 1.0604x; 1.0604x over previous
"""Trainium2 Bass kernel for a pre-LN multi-head attention block.

Model (per batch b): LayerNorm(x) -> QKV -> 16-head attention (dh=64) ->
output projection + bias.

Sharding over 8 NeuronCores: core c handles batch b = c//2 and query
seq-half s = c%2 (all 16 heads, 1024 query rows, full 2048 keys).  K/V
projections are duplicated across the pair, but outputs are disjoint row
slices, so unsharding is a pure host-side concat (no collectives).

The same NEFF runs on every core: the host hands odd cores x with its two
seq halves swapped, so "my queries" are always rows 0:1023 of the local
view.  Attention results are invariant to key/value ordering (softmax sum
and PV sum are permutation-invariant), so the swapped key order on odd
cores changes nothing.

Device-side layout notes:
 - Activations are kept transposed (feature dim on partitions): every
   matmul contracts over the partition axis.
 - Scores are computed directly as S^T [nk, nq]; softmax needs no max
   subtraction (scores ~ N(0,1)), so exp is one ScalarE pass and the
   denominator rides along as a ones-column in the PV matmul (M=65).
 - ln_gamma/ln_beta are folded into the QKV weights host-side; b_out is
   added host-side.
 - QKV weight tiles stream from DRAM per output tile; q/k/V_ext tiles are
   transient, produced per head-pair right before that pair's attention,
   which keeps TensorE densely busy (HAM stays at full clock).
"""

import numpy as np
from ml_dtypes import bfloat16

B, N, D = 4, 2048, 1024
HEADS, DH = 16, 64
SCALE = DH ** -0.5
NCORES = 8
NQ = N // 2                 # 1024 query rows per core
EPS = 1e-5
NT = N // 128               # 16 sequence tiles (LN)
KD = D // 128               # 8 feature tiles
NKT = N // 128              # 16 key tiles
NOB = 3 * D // 128          # 24 qkv output row-tiles (q:0-7, k:8-15, v:16-23)

_cache = {}


def _build():
    import concourse.bass as bass
    import concourse.mybir as mybir
    import concourse.bacc as bacc
    import concourse.tile as tile
    from concourse.masks import make_identity

    f32 = mybir.dt.float32
    bf16 = mybir.dt.bfloat16
    AX = mybir.AxisListType
    ALU = mybir.AluOpType
    ACTF = mybir.ActivationFunctionType

    nc = bacc.Bacc(
        "TRN2",
        target_bir_lowering=False,
        debug=False,
        enable_asserts=True,
        num_devices=NCORES,
    )

    x_d = nc.dram_tensor("x", [N, D], f32, kind="ExternalInput").ap()
    wq_d = nc.dram_tensor("wqkvT", [D, 3 * D], bf16, kind="ExternalInput").ap()
    bias_d = nc.dram_tensor("qkv_bias", [128, NOB], f32, kind="ExternalInput").ap()
    wo_d = nc.dram_tensor("woutT", [D, D], bf16, kind="ExternalInput").ap()
    out_d = nc.dram_tensor("out", [NQ, D], f32, kind="ExternalOutput").ap()

    with tile.TileContext(nc) as tc:
        with (
            tc.tile_pool(name="persist", bufs=1) as P,
            tc.tile_pool(name="ppool", bufs=1, space="PSUM") as PS,
            tc.tile_pool(name="trans", bufs=1) as T,
        ):
            ident = P.tile([128, 128], bf16, name="ident", tag="ident")
            make_identity(nc, ident)
            eps_t = P.tile([128, 1], f32, name="eps_t", tag="eps_t")
            nc.vector.memset(eps_t, EPS)

            bias_sb = P.tile([128, NOB], f32, name="bias_sb", tag="bias_sb")
            nc.sync.dma_start(bias_sb, bias_d)

            wo_sb = []
            for k in range(KD):
                t = P.tile([128, D], bf16, name=f"wo{k}", tag=f"wo{k}")
                nc.sync.dma_start(t, wo_d[k * 128:(k + 1) * 128, :])
                wo_sb.append(t)

            # xnT: transposed normalized activations [d, n] as [128, KD*N]
            xnT = P.tile([128, KD * N], bf16, name="xnT", tag="xnT")
            xnT3 = xnT.rearrange("p (k n) -> p k n", k=KD)
            # normalized attention outputs, transposed: [1024 hd, 1024 nq]
            onormT = []
            for k in range(KD):
                onormT.append(
                    P.tile([128, NQ], bf16, name=f"onormT{k}", tag=f"onormT{k}")
                )

            sq_scr = T.tile([128, D], f32, name="sq_scr", tag="sq", bufs=1)

            # ---- Phase A: LayerNorm + transpose, pipelined over seq tiles
            for nt in range(NT):
                x_t = T.tile([128, D], f32, name=f"x{nt}", tag="x", bufs=2)
                nc.sync.dma_start(x_t, x_d[nt * 128:(nt + 1) * 128, :])
                ssum = T.tile([128, 1], f32, name=f"ss{nt}", tag="ss", bufs=2)
                nc.vector.tensor_reduce(out=ssum, in_=x_t, axis=AX.X, op=ALU.add)
                mean = T.tile([128, 1], f32, name=f"mn{nt}", tag="mn", bufs=2)
                nc.scalar.mul(mean, ssum, 1.0 / D)
                xc = T.tile([128, D], f32, name=f"xc{nt}", tag="xc", bufs=2)
                nc.vector.tensor_scalar_sub(xc, x_t, mean)
                var = T.tile([128, 1], f32, name=f"vr{nt}", tag="vr", bufs=2)
                nc.scalar.activation(sq_scr, xc, ACTF.Square, accum_out=var)
                std = T.tile([128, 1], f32, name=f"st{nt}", tag="st", bufs=2)
                nc.scalar.activation(std, var, ACTF.Sqrt, bias=eps_t, scale=1.0 / D)
                rstd = T.tile([128, 1], f32, name=f"rs{nt}", tag="rs", bufs=2)
                nc.vector.reciprocal(rstd, std)
                xhat = T.tile([128, D], bf16, name=f"xh{nt}", tag="xh", bufs=2)
                nc.vector.tensor_scalar_mul(xhat, xc, rstd)
                for g2 in range(2):
                    tp = PS.tile(
                        [128, 512], bf16, name=f"tp{nt}_{g2}", tag="work", bufs=2
                    )
                    for j in range(4):
                        kd = g2 * 4 + j
                        nc.tensor.transpose(
                            tp[:, j * 128:(j + 1) * 128],
                            xhat[:, kd * 128:(kd + 1) * 128],
                            ident,
                        )
                    dest = xnT3[:, g2 * 4:(g2 + 1) * 4, nt * 128:(nt + 1) * 128]
                    src = tp.rearrange("p (k n) -> p k n", k=4)
                    if (nt + g2) % 2 == 0:
                        nc.vector.tensor_copy(dest, src)
                    else:
                        nc.scalar.copy(dest, src)

            def qkv_tile(ob, ncols, name):
                """Project qkv output row-tile ob over ncols seq columns.
                Streams the weight slice from DRAM; returns a [128, ncols]
                bf16 tile (tag shared by name for rotation)."""
                wts = []
                for k in range(KD):
                    wt = T.tile(
                        [128, 128], bf16, name=f"w{name}_{k}", tag=f"wqs{k}", bufs=3
                    )
                    nc.sync.dma_start(
                        wt, wq_d[k * 128:(k + 1) * 128, ob * 128:(ob + 1) * 128]
                    )
                    wts.append(wt)
                dst = T.tile([128, ncols], bf16, name=f"t{name}", tag=name, bufs=3)
                for c in range(ncols // 512):
                    qp = PS.tile(
                        [128, 512], f32, name=f"qp{name}_{c}", tag="work", bufs=2
                    )
                    for k in range(KD):
                        nc.tensor.matmul(
                            qp,
                            lhsT=wts[k],
                            rhs=xnT3[:, k, c * 512:(c + 1) * 512],
                            start=(k == 0),
                            stop=(k == KD - 1),
                        )
                    dcol = dst[:, c * 512:(c + 1) * 512]
                    if c % 2 == 0:
                        nc.scalar.activation(
                            dcol, qp, ACTF.Identity, bias=bias_sb[:, ob:ob + 1]
                        )
                    else:
                        nc.vector.tensor_scalar_add(dcol, qp, bias_sb[:, ob:ob + 1])
                return dst

            # ---- Phases B+C interleaved per head pair
            for j in range(KD):  # 8 head pairs
                qT_j = qkv_tile(j, NQ, "qT")
                kT_j = qkv_tile(8 + j, N, "kT")
                vT_j = qkv_tile(16 + j, N, "vT")

                # V_ext for the two heads: [nk, dh | ones] blocks per key tile
                ve3s = []
                for h2 in range(2):
                    p0 = h2 * 64
                    id64 = ident[p0:p0 + 64, p0:p0 + 64]
                    ve = T.tile(
                        [128, NKT * 65], bf16, name=f"vx{j}_{h2}", tag="vext", bufs=4
                    )
                    nc.vector.memset(ve, 1.0)
                    ve3 = ve.rearrange("p (k e) -> p k e", e=65)
                    for g2 in range(2):
                        tp = PS.tile(
                            [128, 512], bf16, name=f"vt{j}_{h2}_{g2}",
                            tag="work", bufs=2,
                        )
                        for i8 in range(8):
                            kt = g2 * 8 + i8
                            nc.tensor.transpose(
                                tp[:, i8 * 64:(i8 + 1) * 64],
                                vT_j[p0:p0 + 64, kt * 128:(kt + 1) * 128],
                                id64,
                            )
                        dest = ve3[:, g2 * 8:(g2 + 1) * 8, 0:64]
                        src = tp.rearrange("p (k e) -> p k e", e=64)
                        if (h2 + g2) % 2 == 0:
                            nc.vector.tensor_copy(dest, src)
                        else:
                            nc.scalar.copy(dest, src)
                    ve3s.append(ve3)

                for h2 in range(2):
                    h = 2 * j + h2
                    p0 = h2 * 64
                    ops = PS.tile(
                        [65, NQ], f32, name=f"ops{h}", tag="acc", bufs=2
                    )
                    for kt in range(NKT):
                        sps = PS.tile(
                            [128, NQ], f32, name=f"s{h}_{kt}", tag="work", bufs=2
                        )
                        for c2 in range(2):
                            nc.tensor.matmul(
                                sps[:, c2 * 512:(c2 + 1) * 512],
                                lhsT=kT_j[p0:p0 + 64, kt * 128:(kt + 1) * 128],
                                rhs=qT_j[p0:p0 + 64, c2 * 512:(c2 + 1) * 512],
                                start=True,
                                stop=True,
                            )
                        pt = T.tile(
                            [128, NQ], bf16, name=f"pt{h}_{kt}", tag="pt", bufs=3
                        )
                        nc.scalar.activation(pt, sps, ACTF.Exp, scale=SCALE)
                        for c2 in range(2):
                            nc.tensor.matmul(
                                ops[:, c2 * 512:(c2 + 1) * 512],
                                lhsT=ve3s[h2][:, kt, :],
                                rhs=pt[:, c2 * 512:(c2 + 1) * 512],
                                start=(kt == 0),
                                stop=(kt == NKT - 1),
                            )
                    # normalize by softmax denominator (row 64 = ones . P)
                    rl = T.tile([1, NQ], f32, name=f"rl{h}", tag="rl", bufs=2)
                    nc.vector.reciprocal(rl, ops[64:65, :])
                    rlb = T.tile([64, NQ], f32, name=f"rlb{h}", tag="rlb", bufs=2)
                    nc.gpsimd.partition_broadcast(rlb, rl, channels=64)
                    nc.vector.tensor_mul(
                        onormT[h // 2][p0:p0 + 64, :], ops[0:64, :], rlb
                    )

            # ---- Phase D: output projection [1024 nq, 1024 dm]
            for nt in range(NQ // 128):
                po = T.tile([128, D], f32, name=f"po{nt}", tag="po", bufs=2)
                for c in range(2):
                    pp = PS.tile(
                        [128, 512], f32, name=f"pp{nt}_{c}", tag="work", bufs=2
                    )
                    for kq in range(KD):
                        nc.tensor.matmul(
                            pp,
                            lhsT=onormT[kq][:, nt * 128:(nt + 1) * 128],
                            rhs=wo_sb[kq][:, c * 512:(c + 1) * 512],
                            start=(kq == 0),
                            stop=(kq == KD - 1),
                        )
                    if c == 0:
                        nc.scalar.copy(po[:, c * 512:(c + 1) * 512], pp)
                    else:
                        nc.vector.tensor_copy(po[:, c * 512:(c + 1) * 512], pp)
                nc.sync.dma_start(out_d[nt * 128:(nt + 1) * 128, :], po)

    nc.compile()
    return nc


def _shard_inputs(x, ln_gamma, ln_beta, w_qkv, w_out):
    w_eff = (w_qkv * ln_gamma[None, :]).astype(np.float32)
    wqkvT = np.ascontiguousarray(w_eff.T).astype(bfloat16)          # [1024, 3072]
    bias = (w_qkv.astype(np.float64) @ ln_beta.astype(np.float64))
    bias_2d = np.ascontiguousarray(
        bias.reshape(NOB, 128).T
    ).astype(np.float32)                                            # [128, 24]
    woutT = np.ascontiguousarray(w_out.T).astype(bfloat16)          # [1024, 1024]
    in_maps = []
    for c in range(NCORES):
        b, s = c // 2, c % 2
        xb = np.asarray(x[b], dtype=np.float32)
        if s == 1:
            xb = np.concatenate([xb[NQ:], xb[:NQ]], axis=0)
        in_maps.append({
            "x": np.ascontiguousarray(xb),
            "wqkvT": wqkvT,
            "qkv_bias": bias_2d,
            "woutT": woutT,
        })
    return in_maps


def kernel(x, ln_gamma, ln_beta, w_qkv, w_out, b_out, _trace=False):
    from concourse import bass_utils

    x = np.asarray(x, dtype=np.float32)
    ln_gamma = np.asarray(ln_gamma, dtype=np.float32)
    ln_beta = np.asarray(ln_beta, dtype=np.float32)
    w_qkv = np.asarray(w_qkv, dtype=np.float32)
    w_out = np.asarray(w_out, dtype=np.float32)
    b_out = np.asarray(b_out, dtype=np.float32)

    if "nc" not in _cache:
        _cache["nc"] = _build()
    nc = _cache["nc"]

    in_maps = _shard_inputs(x, ln_gamma, ln_beta, w_qkv, w_out)
    res = bass_utils.run_bass_kernel_spmd(
        nc, in_maps, core_ids=list(range(NCORES)), trace=_trace
    )
    out = np.empty((B, N, D), dtype=np.float32)
    for b in range(B):
        out[b, :NQ] = np.asarray(res.results[2 * b]["out"])
        out[b, NQ:] = np.asarray(res.results[2 * b + 1]["out"])
    out += b_out[None, None, :]
    _cache["last_result"] = res
    return out
